# revision 1
# baseline (speedup 1.0000x reference)
"""Trainium2 Bass kernel for CartesianLoss.

Loss = mean_n min_perm mean_i ||polar2cart(target_i) - polar2cart(pred_perm(i))||_2

Strategy: pure data parallelism over the batch (N=131072) across 8 cores.
Each core handles 16384 samples laid out as (128 partitions, 128 samples).
Per sample we build the 5x5 distance matrix D[i,j] with fat broadcast-AP
vector ops, then compute min over the 120 permutations with a
meet-in-the-middle decomposition:
    F01[a,b] = min assignment of rows {0,1} to preds {a,b}
    F23[a,b] = same for rows {2,3}
    g3[T]    = min assignment of rows {2,3,4} to pred triple T
    ans      = min over pairs W of F01[W] + g3[complement(W)]
(complement of the k-th lex pair is the (9-k)-th lex triple, so the combine
is a reversed-stride access pattern).

Everything downstream of the polar->cartesian conversion runs in fp16
(values bounded by ~7e3, fp16 rounding ~6e-4 relative) which unlocks the
DVE 2x packed perf mode on the fat elementwise ops.

Per-core output: per-partition fp32 sums of the per-sample min; host reduces.
"""

import contextlib

import numpy as np

import concourse.bass as bass
import concourse.bacc as bacc
import concourse.tile as tile
from concourse import mybir

N = 131072
M = 5
NCORES = 8
NPC = N // NCORES          # samples per core
P = 128                    # partitions
FS = NPC // P              # samples per partition (128)
HALF_PI = 1.5707963267948966

F32 = mybir.dt.float32
F16 = mybir.dt.float16
TT = mybir.AluOpType

LAST_EXEC_TIME_NS = None
LAST_RESULTS = None
TRACE = False

_CACHED_NC = None


def _ap(t, offset_elems, dims):
    """Manual free-dim AP on tile t: dims = [[step,count],...] (elements)."""
    full = t[:]
    return bass.AP(
        tensor=full.tensor,
        offset=full.offset + offset_elems,
        ap=[full.ap[0]] + [list(d) for d in dims],
    )


def build_bass(loop_iters=None):
    """Build the SPMD program. loop_iters (benchmark only): wrap the whole
    body in an on-device For_i so one launch runs the pipeline many times."""
    nc = bacc.Bacc(
        "TRN2", target_bir_lowering=False, debug=False, num_devices=NCORES
    )
    # Register pi/2 as a preamble const AP (same mechanism as Bass's built-in
    # 0.0/1.0) so activation(bias=HALF_PI) needs no runtime semaphore wait.
    hpi_t = nc.alloc_sbuf_tensor("const-float32-hpi", [P, 1], F32)
    nc.gpsimd.memset(hpi_t.ap(), HALF_PI)
    nc.const_aps.aps[(F32, HALF_PI)] = hpi_t.ap()
    nc.all_engine_barrier()

    ta_d = nc.dram_tensor("targets_angle", [NPC, M], F32, kind="ExternalInput")
    pa_d = nc.dram_tensor("predictions_angle", [NPC, M], F32, kind="ExternalInput")
    td_d = nc.dram_tensor("targets_distance", [NPC, M], F32, kind="ExternalInput")
    pd_d = nc.dram_tensor("predictions_distance", [NPC, M], F32, kind="ExternalInput")
    out_d = nc.dram_tensor("partials", [P, 1], F32, kind="ExternalOutput")

    with tile.TileContext(nc) as tc:
        with contextlib.ExitStack() as stack:
            if loop_iters is not None:
                stack.enter_context(tc.For_i(0, loop_iters, 1))
            pool = stack.enter_context(tc.tile_pool(name="main", bufs=1))

            # ---- load inputs: (128, FS, 5), sample-major per partition ----
            ins = {}
            for name, dram in (
                ("ta", ta_d), ("pa", pa_d), ("td", td_d), ("pd", pd_d),
            ):
                t = pool.tile([P, FS, M], F32, tag=f"in_{name}")
                src = dram[:].rearrange("(p s) m -> p s m", p=P)
                nc.sync.dma_start(out=t[:], in_=src)
                ins[name] = t

            # ---- trig (ACT): cos x = sin(x + pi/2) ----
            ct = pool.tile([P, FS, M], F32, tag="ct")
            st = pool.tile([P, FS, M], F32, tag="st")
            cp = pool.tile([P, FS, M], F32, tag="cp")
            sp = pool.tile([P, FS, M], F32, tag="sp")
            SIN = mybir.ActivationFunctionType.Sin
            nc.scalar.activation(ct[:], ins["ta"][:], SIN, bias=HALF_PI)
            nc.scalar.activation(st[:], ins["ta"][:], SIN)
            nc.scalar.activation(cp[:], ins["pa"][:], SIN, bias=HALF_PI)
            nc.scalar.activation(sp[:], ins["pa"][:], SIN)

            # ---- coords (DVE): fp32 mul (strided in), fp16 out src-major
            # (128, 5, FS) so the downstream fat subs get contiguous inner
            # dim -> DVE 2x packed mode.
            TX = pool.tile([P, M, FS], F16, tag="TX")
            TY = pool.tile([P, M, FS], F16, tag="TY")
            PX = pool.tile([P, M, FS], F16, tag="PX")
            PY = pool.tile([P, M, FS], F16, tag="PY")
            tdv = ins["td"][:].transpose([0, 2, 1])
            pdv = ins["pd"][:].transpose([0, 2, 1])
            nc.vector.tensor_tensor(TX[:], ct[:].transpose([0, 2, 1]), tdv, TT.mult)
            nc.vector.tensor_tensor(TY[:], st[:].transpose([0, 2, 1]), tdv, TT.mult)
            nc.vector.tensor_tensor(PX[:], cp[:].transpose([0, 2, 1]), pdv, TT.mult)
            nc.vector.tensor_tensor(PY[:], sp[:].transpose([0, 2, 1]), pdv, TT.mult)

            # views of src-major coord tiles as (i, j, sample), contiguous inner
            def tview(t):   # index i (true source), broadcast over j
                return _ap(t, 0, [[FS, M], [0, M], [1, FS]])

            def pview(t):   # index j (pred source), broadcast over i
                return _ap(t, 0, [[0, M], [FS, M], [1, FS]])

            # ---- distance matrix D[i,j] (128, 5, 5, FS) fp16 ----
            DX = pool.tile([P, M, M, FS], F16, tag="wA")
            DY = pool.tile([P, M, M, FS], F16, tag="wB")
            nc.vector.tensor_tensor(DX[:], tview(TX), pview(PX), TT.subtract)
            nc.vector.tensor_tensor(DY[:], tview(TY), pview(PY), TT.subtract)
            DX2 = pool.tile([P, M, M, FS], F16, tag="DX2")
            DY2 = pool.tile([P, M, M, FS], F16, tag="DY2")
            nc.vector.tensor_tensor(DX2[:], DX[:], DX[:], TT.mult)
            nc.vector.tensor_tensor(DY2[:], DY[:], DY[:], TT.mult)
            nc.vector.tensor_tensor(DX2[:], DX2[:], DY2[:], TT.add)  # d^2
            D = pool.tile([P, M, M, FS], F16, tag="D")
            nc.scalar.activation(D[:], DX2[:], mybir.ActivationFunctionType.Sqrt)

            # row views (128, 5, FS)
            D0, D1, D2r, D3r, D4 = (D[:, i, :, :] for i in range(5))

            # ---- pair stage ----
            # G[a,b] = D[r0,a] + D[r1,b] (25-dense), then fold the lower
            # triangle into the upper with in-place mins: for a<b,
            # F[a,b] = min(G[a,b], G[b,a]). Downstream only reads a<b slots.
            G = pool.tile([P, M, M, FS], F16, tag="G")
            F01 = pool.tile([P, M, M, FS], F16, tag="wA")  # reuse DX slot
            F23 = pool.tile([P, M, M, FS], F16, tag="wB")  # reuse DY slot
            for F, Ra, Rb in ((F01, D0, D1), (F23, D2r, D3r)):
                nc.vector.tensor_tensor(
                    G[:],
                    Ra[:, :, None, :].broadcast_to((P, M, M, FS)),
                    Rb[:, None, :, :].broadcast_to((P, M, M, FS)),
                    TT.add,
                )
                for a in range(4):
                    n = 4 - a
                    nc.vector.tensor_tensor(
                        _ap(F, (6 * a + 1) * FS, [[FS, n], [1, FS]]),
                        _ap(G, (6 * a + 1) * FS, [[FS, n], [1, FS]]),
                        _ap(G, ((a + 1) * M + a) * FS, [[M * FS, n], [1, FS]]),
                        TT.min,
                    )

            # ---- g3 arms over lex triples (q<r<t), ranks 0..9 ----
            At = pool.tile([P, 10, FS], F16, tag="At")
            Ar = pool.tile([P, 10, FS], F16, tag="Ar")
            Aq = pool.tile([P, 10, FS], F16, tag="Aq")

            def f23_flat(offset_elems, dims):
                return _ap(F23, offset_elems, dims)

            # arm_t: j = t (largest). At[T] = F23[q,r] + D4[t]
            nc.vector.tensor_tensor(  # (0,1), t in {2,3,4} -> ranks 0..2
                At[:, 0:3, :],
                f23_flat(1 * FS, [[0, 3], [1, FS]]),
                D4[:, 2:5, :],
                TT.add,
            )
            nc.vector.tensor_tensor(  # pairs (0,2),(1,2), t in {3,4} -> ranks {3,4},{6,7}
                _ap(At, 3 * FS, [[3 * FS, 2], [FS, 2], [1, FS]]),
                f23_flat(2 * FS, [[5 * FS, 2], [0, 2], [1, FS]]),
                _ap(D4, 3 * FS, [[0, 2], [FS, 2], [1, FS]]),
                TT.add,
            )
            nc.vector.tensor_tensor(  # pairs (0,3),(1,3), t=4 -> ranks {5},{8}
                _ap(At, 5 * FS, [[3 * FS, 2], [1, FS]]),
                f23_flat(3 * FS, [[5 * FS, 2], [1, FS]]),
                D4[:, 4, None, :].broadcast_to((P, 2, FS)),
                TT.add,
            )
            nc.vector.tensor_tensor(  # pair (2,3), t=4 -> rank 9
                At[:, 9:10, :],
                f23_flat(13 * FS, [[0, 1], [1, FS]]),
                D4[:, 4:5, :],
                TT.add,
            )

            # arm_r: j = r (middle). Ar[T] = F23[q,t] + D4[r]
            nc.vector.tensor_tensor(  # r=1: (0,t) t in {2,3,4} -> ranks 0..2
                Ar[:, 0:3, :],
                f23_flat(2 * FS, [[FS, 3], [1, FS]]),
                D4[:, 1, None, :].broadcast_to((P, 3, FS)),
                TT.add,
            )
            nc.vector.tensor_tensor(  # r=2: q in {0,1}, t in {3,4} -> ranks {3,4},{6,7}
                _ap(Ar, 3 * FS, [[3 * FS, 2], [FS, 2], [1, FS]]),
                f23_flat(3 * FS, [[5 * FS, 2], [FS, 2], [1, FS]]),
                D4[:, 2, None, None, :].broadcast_to((P, 2, 2, FS)),
                TT.add,
            )
            nc.vector.tensor_tensor(  # r=3: q in {0,1}, t=4 -> ranks {5},{8}
                _ap(Ar, 5 * FS, [[3 * FS, 2], [1, FS]]),
                f23_flat(4 * FS, [[5 * FS, 2], [1, FS]]),
                D4[:, 3, None, :].broadcast_to((P, 2, FS)),
                TT.add,
            )
            nc.vector.tensor_tensor(  # r=3, q=2 (triple 234) -> rank 9
                Ar[:, 9:10, :],
                f23_flat(14 * FS, [[0, 1], [1, FS]]),
                D4[:, 3:4, :],
                TT.add,
            )

            # arm_q: j = q (smallest). Aq[T] = F23[r,t] + D4[q]
            nc.vector.tensor_tensor(  # q=0, r=1: t in {2,3,4} -> ranks 0..2
                Aq[:, 0:3, :],
                f23_flat(7 * FS, [[FS, 3], [1, FS]]),
                D4[:, 0, None, :].broadcast_to((P, 3, FS)),
                TT.add,
            )
            nc.vector.tensor_tensor(  # r=2: q in {0,1}, t in {3,4} -> ranks {3,4},{6,7}
                _ap(Aq, 3 * FS, [[3 * FS, 2], [FS, 2], [1, FS]]),
                f23_flat(13 * FS, [[0, 2], [FS, 2], [1, FS]]),
                _ap(D4, 0, [[FS, 2], [0, 2], [1, FS]]),
                TT.add,
            )
            nc.vector.tensor_tensor(  # r=3, q in {0,1}, t=4 -> ranks {5},{8}
                _ap(Aq, 5 * FS, [[3 * FS, 2], [1, FS]]),
                f23_flat(19 * FS, [[0, 2], [1, FS]]),
                D4[:, 0:2, :],
                TT.add,
            )
            nc.vector.tensor_tensor(  # q=2, r=3, t=4 -> rank 9
                Aq[:, 9:10, :],
                f23_flat(19 * FS, [[0, 1], [1, FS]]),
                D4[:, 2:3, :],
                TT.add,
            )

            g3 = pool.tile([P, 10, FS], F16, tag="g3")
            nc.vector.tensor_tensor(g3[:], Ar[:], Aq[:], TT.min)
            nc.vector.tensor_tensor(g3[:], g3[:], At[:], TT.min)

            # ---- combine: ans[k] = F01[pair k] + g3[9-k] ----
            ans = pool.tile([P, 10, FS], F16, tag="ans")
            nc.vector.tensor_tensor(  # a=0: pairs (0,1..4) -> ranks 0..3
                ans[:, 0:4, :],
                _ap(F01, 1 * FS, [[FS, 4], [1, FS]]),
                _ap(g3, 9 * FS, [[-FS, 4], [1, FS]]),
                TT.add,
            )
            nc.vector.tensor_tensor(  # a=1: pairs (1,2..4) -> ranks 4..6
                ans[:, 4:7, :],
                _ap(F01, 7 * FS, [[FS, 3], [1, FS]]),
                _ap(g3, 5 * FS, [[-FS, 3], [1, FS]]),
                TT.add,
            )
            nc.vector.tensor_tensor(  # a=2: pairs (2,3..4) -> ranks 7..8
                ans[:, 7:9, :],
                _ap(F01, 13 * FS, [[FS, 2], [1, FS]]),
                _ap(g3, 2 * FS, [[-FS, 2], [1, FS]]),
                TT.add,
            )
            nc.vector.tensor_tensor(  # a=3: pair (3,4) -> rank 9
                ans[:, 9:10, :],
                _ap(F01, 19 * FS, [[0, 1], [1, FS]]),
                g3[:, 0:1, :],
                TT.add,
            )

            # ---- per-sample min over the 10 combine slots, then sum ----
            res = pool.tile([P, FS], F32, tag="res")
            nc.vector.tensor_reduce(
                res[:], ans[:].transpose([0, 2, 1]), mybir.AxisListType.X, TT.min
            )
            part = pool.tile([P, 1], F32, tag="part")
            nc.vector.tensor_reduce(part[:], res[:], mybir.AxisListType.X, TT.add)
            nc.sync.dma_start(out=out_d[:], in_=part[:])

    nc.compile()
    return nc


_CACHED_RUNNER = None


def _make_runner():
    """Build the program once and wrap it in a cached jitted shard_map
    callable (mirrors bass2jax.run_bass_via_pjrt, minus per-call retracing)."""
    import jax
    from jax.sharding import Mesh, NamedSharding, PartitionSpec
    from jax.experimental.shard_map import shard_map
    from concourse.bass2jax import (
        _bass_exec_p, install_neuronx_cc_hook, partition_id_tensor,
    )

    nc = build_bass()
    install_neuronx_cc_hook()
    partition_name = nc.partition_id_tensor.name if nc.partition_id_tensor else None
    in_names, out_names, out_avals, zero_outs = [], [], [], []
    for alloc in nc.m.functions[0].allocations:
        if not isinstance(alloc, mybir.MemoryLocationSet):
            continue
        name = alloc.memorylocations[0].name
        if alloc.kind == "ExternalInput":
            if name != partition_name:
                in_names.append(name)
        elif alloc.kind == "ExternalOutput":
            shape = tuple(alloc.tensor_shape)
            dtype = mybir.dt.np(alloc.dtype)
            out_names.append(name)
            out_avals.append(jax.core.ShapedArray(shape, dtype))
            zero_outs.append(np.zeros(shape, dtype))
    n_params = len(in_names)
    all_in_names = in_names + out_names
    if partition_name is not None:
        all_in_names = all_in_names + [partition_name]

    def _body(*args):
        operands = list(args)
        if partition_name is not None:
            operands.append(partition_id_tensor())
        return tuple(_bass_exec_p.bind(
            *operands,
            out_avals=tuple(out_avals),
            in_names=tuple(all_in_names),
            out_names=tuple(out_names),
            lowering_input_output_aliases=(),
            sim_require_finite=True,
            sim_require_nnan=True,
            nc=nc,
        ))

    devices = jax.devices()[:NCORES]
    mesh = Mesh(np.asarray(devices), ("core",))
    in_specs = (PartitionSpec("core"),) * (n_params + len(out_names))
    out_specs = (PartitionSpec("core"),) * len(out_names)
    fn = jax.jit(
        shard_map(_body, mesh=mesh, in_specs=in_specs, out_specs=out_specs,
                  check_rep=False),
        keep_unused=True,
    )
    sharding = NamedSharding(mesh, PartitionSpec("core"))
    concat_zeros = [
        np.zeros((NCORES * z.shape[0], *z.shape[1:]), z.dtype) for z in zero_outs
    ]
    zeros_dev = [jax.device_put(z, sharding) for z in concat_zeros]

    def run(inputs_by_name):
        import jax as _jax
        args = [
            _jax.device_put(
                np.ascontiguousarray(inputs_by_name[nm], np.float32), sharding
            )
            for nm in in_names
        ]
        outs = fn(*args, *zeros_dev)
        return {nm: np.asarray(outs[i]) for i, nm in enumerate(out_names)}

    return run


def kernel(predictions_angle, targets_angle, predictions_distance, targets_distance):
    global _CACHED_RUNNER
    if _CACHED_RUNNER is None:
        _CACHED_RUNNER = _make_runner()
    out = _CACHED_RUNNER({
        "targets_angle": targets_angle,
        "predictions_angle": predictions_angle,
        "targets_distance": targets_distance,
        "predictions_distance": predictions_distance,
    })
    # global output is the 8 cores' (128,1) partials stacked on axis 0
    total = out["partials"].astype(np.float64).sum()
    return np.asarray(total / N / M, dtype=np.float32)



# revision 9
# speedup vs baseline: 1.1542x; 1.1542x over previous
"""Trainium2 Bass kernel for CartesianLoss (v2, chunked multi-engine pipeline).

Loss = mean_n min_perm mean_i ||polar2cart(target_i) - polar2cart(pred_perm(i))||_2

Strategy: pure data parallelism over the batch (N=131072) across 8 cores.
Each core handles 16384 samples laid out as (128 partitions, 128 samples).

v2 changes vs v1:
- Host packs inputs chunk-major, source-major, fp16: ang/dst dram tensors of
  shape [P, NCH, 2, M, CS] so every device op reads contiguous fp16 (DVE 2x
  packed mode everywhere) and no on-device transposes or fp32 ops remain.
- Work is split into NCH sample-chunks that pipeline across engines:
  ACT (sin/cos, squares, sqrt), DVE (products, outer-subs, pair/arm algebra),
  optionally GpSimd for offloaded stages.
- All sins for all chunks are emitted before the first Square/Sqrt so the
  ACT table set switches exactly once.
- Per-sample min + batch-sum fused via tensor_tensor_reduce, whose scalar
  init chains the per-chunk partial sums.

Assignment min over 120 perms uses the meet-in-the-middle decomposition
(unchanged from v1): F01/F23 pair mins via dense 5x5 outer-sum + triangle
fold, g3 triples via 3 arms, combine with reversed-rank access.
"""

import contextlib

import numpy as np

import concourse.bass as bass
import concourse.bacc as bacc
import concourse.tile as tile
from concourse import mybir

N = 131072
M = 5
NCORES = 8
NPC = N // NCORES          # samples per core
P = 128                    # partitions
FS = NPC // P              # samples per partition (128)
HALF_PI = 1.5707963267948966

F32 = mybir.dt.float32
F16 = mybir.dt.float16
TT = mybir.AluOpType
AFT = mybir.ActivationFunctionType

# --- tunables -------------------------------------------------------------
NCH = 2                    # sample chunks per partition (divides FS)
SQ_ENGINE = "act"          # 'act' | 'dve'   (squares of dx/dy)
ADD_ENGINE = "dve"         # 'dve' | 'gp'    (d2 = dx2 + dy2)
ARMS_ENGINE = "dve"        # 'dve' | 'gp' | 'split'
G3_ENGINE = "dve"          # 'dve' | 'gp'    (3-way arm min)
FINAL_MODE = "reduce"      # 'ttr' | 'reduce' (fused min+sum vs plain reduce)

LAST_EXEC_TIME_NS = None
TRACE = False


def _ap(t, offset_elems, dims):
    """Manual free-dim AP on tile t: dims = [[step,count],...] (elements)."""
    full = t[:]
    return bass.AP(
        tensor=full.tensor,
        offset=full.offset + offset_elems,
        ap=[full.ap[0]] + [list(d) for d in dims],
    )


def build_bass(loop_iters=None, nch=None, sq_engine=None, add_engine=None,
               arms_engine=None, g3_engine=None, final_mode=None):
    nch = NCH if nch is None else nch
    sq_engine = SQ_ENGINE if sq_engine is None else sq_engine
    add_engine = ADD_ENGINE if add_engine is None else add_engine
    arms_engine = ARMS_ENGINE if arms_engine is None else arms_engine
    g3_engine = G3_ENGINE if g3_engine is None else g3_engine
    final_mode = FINAL_MODE if final_mode is None else final_mode
    CS = FS // nch
    assert FS % nch == 0

    nc = bacc.Bacc(
        "TRN2", target_bir_lowering=False, debug=False, num_devices=NCORES
    )
    # Register pi/2 as a preamble const AP so activation(bias=HALF_PI) needs
    # no runtime semaphore wait.
    hpi_t = nc.alloc_sbuf_tensor("const-float32-hpi", [P, 1], F32)
    nc.gpsimd.memset(hpi_t.ap(), HALF_PI)
    nc.const_aps.aps[(F32, HALF_PI)] = hpi_t.ap()
    nc.all_engine_barrier()

    # ang[p, c, 0, m, s] = targets_angle, [p, c, 1, m, s] = predictions_angle
    # dst likewise for distances; packed host-side, fp16, chunk-major.
    ang_d = nc.dram_tensor("ang", [P, nch, 2, M, CS], F16, kind="ExternalInput")
    dst_d = nc.dram_tensor("dst", [P, nch, 2, M, CS], F16, kind="ExternalInput")
    out_d = nc.dram_tensor("partials", [P, 1], F32, kind="ExternalOutput")

    gp = nc.gpsimd
    dve = nc.vector
    eng = {"dve": dve, "gp": gp}

    with tile.TileContext(nc) as tc:
        with contextlib.ExitStack() as stack:
            if loop_iters is not None:
                stack.enter_context(tc.For_i(0, loop_iters, 1))
            pool = stack.enter_context(tc.tile_pool(name="main", bufs=1))

            ang = [pool.tile([P, 2, M, CS], F16, name="t", tag=f"ang{c}") for c in range(nch)]
            dst = [pool.tile([P, 2, M, CS], F16, name="t", tag=f"dst{c}") for c in range(nch)]
            SIN = [pool.tile([P, 2, M, CS], F16, name="t", tag=f"sin{c}") for c in range(nch)]
            COS = [pool.tile([P, 2, M, CS], F16, name="t", tag=f"cos{c}") for c in range(nch)]
            CRX = [pool.tile([P, 2, M, CS], F16, name="t", tag=f"crx{c}") for c in range(nch)]
            CRY = [pool.tile([P, 2, M, CS], F16, name="t", tag=f"cry{c}") for c in range(nch)]
            DXY = [pool.tile([P, 2, M, M, CS], F16, name="t", tag=f"dxy{c}") for c in range(nch)]
            SQ = [pool.tile([P, 2, M, M, CS], F16, name="t", tag=f"sq{c}") for c in range(nch)]
            D2 = [pool.tile([P, M * M, CS], F16, name="t", tag=f"d2{c}") for c in range(nch)]
            D = [pool.tile([P, M * M, CS], F16, name="t", tag=f"d{c}") for c in range(nch)]
            G01 = [pool.tile([P, M, M, CS], F16, name="t", tag=f"g01_{c}") for c in range(nch)]
            G23 = [pool.tile([P, M, M, CS], F16, name="t", tag=f"g23_{c}") for c in range(nch)]
            At = [pool.tile([P, 10, CS], F16, name="t", tag=f"at{c}") for c in range(nch)]
            Ar = [pool.tile([P, 10, CS], F16, name="t", tag=f"ar{c}") for c in range(nch)]
            Aq = [pool.tile([P, 10, CS], F16, name="t", tag=f"aq{c}") for c in range(nch)]
            G3 = [pool.tile([P, 10, CS], F16, name="t", tag=f"g3_{c}") for c in range(nch)]
            ANS = [pool.tile([P, 10, CS], F16, name="t", tag=f"ans{c}") for c in range(nch)]
            T1 = [pool.tile([P, M, CS], F16, name="t", tag=f"t1_{c}") for c in range(nch)]
            T2 = [pool.tile([P, 2, CS], F16, name="t", tag=f"t2_{c}") for c in range(nch)]
            T3 = [pool.tile([P, 1, CS], F16, name="t", tag=f"t3_{c}") for c in range(nch)]
            TD = [pool.tile([P, CS], F16, name="t", tag=f"td_{c}") for c in range(nch)]
            ACC = [pool.tile([P, 1], F32, name="t", tag=f"acc{c}") for c in range(nch)]
            RES = pool.tile([P, nch, CS], F32, name="t", tag="res")
            PART = pool.tile([P, 1], F32, name="t", tag="part")

            # ---- DMA: all chunks up front (rings run async) ----
            for c in range(nch):
                nc.sync.dma_start(out=ang[c][:], in_=ang_d[:, c])
                nc.sync.dma_start(out=dst[c][:], in_=dst_d[:, c])

            # ---- ACT: all trig first (one table set), then sq/sqrt ----
            for c in range(nch):
                nc.scalar.activation(SIN[c][:], ang[c][:], AFT.Sin)
                nc.scalar.activation(COS[c][:], ang[c][:], AFT.Sin, bias=HALF_PI)

            # ---- DVE: coords + outer subs per chunk ----
            for c in range(nch):
                dve.tensor_tensor(CRX[c][:], COS[c][:], dst[c][:], TT.mult)
                dve.tensor_tensor(CRY[c][:], SIN[c][:], dst[c][:], TT.mult)
                for h, CR in ((0, CRX[c]), (1, CRY[c])):
                    dve.tensor_tensor(
                        DXY[c][:, h],
                        CR[:, 0, :, None, :].broadcast_to((P, M, M, CS)),
                        CR[:, 1, None, :, :].broadcast_to((P, M, M, CS)),
                        TT.subtract,
                    )

            # ---- squares (ACT or DVE), d2 add, sqrt ----
            for c in range(nch):
                if sq_engine == "act":
                    nc.scalar.activation(SQ[c][:], DXY[c][:], AFT.Square)
                else:
                    dve.tensor_tensor(SQ[c][:], DXY[c][:], DXY[c][:], TT.mult)
                eng[add_engine].tensor_tensor(
                    D2[c][:], SQ[c][:, 0], SQ[c][:, 1], TT.add
                )
                nc.scalar.activation(D[c][:], D2[c][:], AFT.Sqrt)

            # ---- pair stage: dense outer-sum G then triangle fold ----
            def rowv(c, i):  # D row i as [P, M(j), CS]
                return D[c][:, 5 * i:5 * i + 5, :]

            for c in range(nch):
                for Gt, r0, r1 in ((G01[c], 0, 1), (G23[c], 2, 3)):
                    dve.tensor_tensor(
                        Gt[:],
                        rowv(c, r0)[:, :, None, :].broadcast_to((P, M, M, CS)),
                        rowv(c, r1)[:, None, :, :].broadcast_to((P, M, M, CS)),
                        TT.add,
                    )
                    for a in range(4):
                        n = 4 - a
                        dve.tensor_tensor(
                            _ap(Gt, (6 * a + 1) * CS, [[CS, n], [1, CS]]),
                            _ap(Gt, (6 * a + 1) * CS, [[CS, n], [1, CS]]),
                            _ap(Gt, ((a + 1) * M + a) * CS, [[M * CS, n], [1, CS]]),
                            TT.min,
                        )

            # ---- arms + g3 + combine + min-tree per chunk ----
            def emit_arms(c, e_t, e_r, e_q):
                F23 = G23[c]
                Dt = D[c]
                D4o = 20 * CS  # offset of row 4 in D

                def d4(j0, dims):
                    return _ap(Dt, D4o + j0 * CS, dims)

                # arm_t: j = t (largest). At[T] = F23[q,r] + D4[t]
                e_t.tensor_tensor(
                    At[c][:, 0:3, :],
                    _ap(F23, 1 * CS, [[0, 3], [1, CS]]),
                    d4(2, [[CS, 3], [1, CS]]),
                    TT.add,
                )
                e_t.tensor_tensor(
                    _ap(At[c], 3 * CS, [[3 * CS, 2], [CS, 2], [1, CS]]),
                    _ap(F23, 2 * CS, [[5 * CS, 2], [0, 2], [1, CS]]),
                    d4(3, [[0, 2], [CS, 2], [1, CS]]),
                    TT.add,
                )
                e_t.tensor_tensor(
                    _ap(At[c], 5 * CS, [[3 * CS, 2], [1, CS]]),
                    _ap(F23, 3 * CS, [[5 * CS, 2], [1, CS]]),
                    d4(4, [[0, 2], [1, CS]]),
                    TT.add,
                )
                e_t.tensor_tensor(
                    At[c][:, 9:10, :],
                    _ap(F23, 13 * CS, [[0, 1], [1, CS]]),
                    d4(4, [[0, 1], [1, CS]]),
                    TT.add,
                )
                # arm_r: j = r (middle). Ar[T] = F23[q,t] + D4[r]
                e_r.tensor_tensor(
                    Ar[c][:, 0:3, :],
                    _ap(F23, 2 * CS, [[CS, 3], [1, CS]]),
                    d4(1, [[0, 3], [1, CS]]),
                    TT.add,
                )
                e_r.tensor_tensor(
                    _ap(Ar[c], 3 * CS, [[3 * CS, 2], [CS, 2], [1, CS]]),
                    _ap(F23, 3 * CS, [[5 * CS, 2], [CS, 2], [1, CS]]),
                    d4(2, [[0, 2], [0, 2], [1, CS]]),
                    TT.add,
                )
                e_r.tensor_tensor(
                    _ap(Ar[c], 5 * CS, [[3 * CS, 2], [1, CS]]),
                    _ap(F23, 4 * CS, [[5 * CS, 2], [1, CS]]),
                    d4(3, [[0, 2], [1, CS]]),
                    TT.add,
                )
                e_r.tensor_tensor(
                    Ar[c][:, 9:10, :],
                    _ap(F23, 14 * CS, [[0, 1], [1, CS]]),
                    d4(3, [[0, 1], [1, CS]]),
                    TT.add,
                )
                # arm_q: j = q (smallest). Aq[T] = F23[r,t] + D4[q]
                e_q.tensor_tensor(
                    Aq[c][:, 0:3, :],
                    _ap(F23, 7 * CS, [[CS, 3], [1, CS]]),
                    d4(0, [[0, 3], [1, CS]]),
                    TT.add,
                )
                e_q.tensor_tensor(
                    _ap(Aq[c], 3 * CS, [[3 * CS, 2], [CS, 2], [1, CS]]),
                    _ap(F23, 13 * CS, [[0, 2], [CS, 2], [1, CS]]),
                    d4(0, [[CS, 2], [0, 2], [1, CS]]),
                    TT.add,
                )
                e_q.tensor_tensor(
                    _ap(Aq[c], 5 * CS, [[3 * CS, 2], [1, CS]]),
                    _ap(F23, 19 * CS, [[0, 2], [1, CS]]),
                    d4(0, [[CS, 2], [1, CS]]),
                    TT.add,
                )
                e_q.tensor_tensor(
                    Aq[c][:, 9:10, :],
                    _ap(F23, 19 * CS, [[0, 1], [1, CS]]),
                    d4(2, [[0, 1], [1, CS]]),
                    TT.add,
                )

            if arms_engine == "split":
                arm_engines = (gp, gp, dve)
            else:
                arm_engines = (eng[arms_engine],) * 3
            g3e = eng[g3_engine]

            for c in range(nch):
                emit_arms(c, *arm_engines)
                g3e.tensor_tensor(G3[c][:], Ar[c][:], Aq[c][:], TT.min)
                g3e.tensor_tensor(G3[c][:], G3[c][:], At[c][:], TT.min)

            for c in range(nch):
                F01 = G01[c]
                # combine: ans[k] = F01[pair k] + g3[9-k]
                dve.tensor_tensor(
                    ANS[c][:, 0:4, :],
                    _ap(F01, 1 * CS, [[CS, 4], [1, CS]]),
                    _ap(G3[c], 9 * CS, [[-CS, 4], [1, CS]]),
                    TT.add,
                )
                dve.tensor_tensor(
                    ANS[c][:, 4:7, :],
                    _ap(F01, 7 * CS, [[CS, 3], [1, CS]]),
                    _ap(G3[c], 5 * CS, [[-CS, 3], [1, CS]]),
                    TT.add,
                )
                dve.tensor_tensor(
                    ANS[c][:, 7:9, :],
                    _ap(F01, 13 * CS, [[CS, 2], [1, CS]]),
                    _ap(G3[c], 2 * CS, [[-CS, 2], [1, CS]]),
                    TT.add,
                )
                dve.tensor_tensor(
                    ANS[c][:, 9:10, :],
                    _ap(F01, 19 * CS, [[0, 1], [1, CS]]),
                    G3[c][:, 0:1, :],
                    TT.add,
                )
                # min tree over the 10 slots, fused batch-sum on the last op
                dve.tensor_tensor(
                    T1[c][:], ANS[c][:, 0:5, :], ANS[c][:, 5:10, :], TT.min
                )
                dve.tensor_tensor(
                    T2[c][:], T1[c][:, 0:2, :], T1[c][:, 2:4, :], TT.min
                )
                dve.tensor_tensor(
                    T3[c][:], T2[c][:, 0:1, :], T2[c][:, 1:2, :], TT.min
                )
                if final_mode == "ttr":
                    dve.tensor_tensor_reduce(
                        out=TD[c][:],
                        in0=T3[c][:, 0, :],
                        in1=T1[c][:, 4, :],
                        scale=1.0,
                        scalar=(0.0 if c == 0 else ACC[c - 1][:]),
                        op0=TT.min,
                        op1=TT.add,
                        accum_out=ACC[c][:],
                    )
                else:
                    dve.tensor_tensor(
                        RES[:, c], T3[c][:, 0, :], T1[c][:, 4, :], TT.min
                    )

            if final_mode == "ttr":
                nc.sync.dma_start(out=out_d[:], in_=ACC[nch - 1][:])
            else:
                dve.tensor_reduce(
                    PART[:], _ap(RES, 0, [[1, nch * CS]]),
                    mybir.AxisListType.X, TT.add,
                )
                nc.sync.dma_start(out=out_d[:], in_=PART[:])

    nc.compile()
    return nc


_CACHED_RUNNER = None


def _pack_pair(a, b, nch):
    """a, b: (N, M) f32 -> (NCORES*P, nch, 2, M, CS) f16, chunk-major."""
    CS = FS // nch
    a5 = np.asarray(a, np.float32).reshape(NCORES * P, nch, CS, M)
    b5 = np.asarray(b, np.float32).reshape(NCORES * P, nch, CS, M)
    out = np.empty((NCORES * P, nch, 2, M, CS), np.float16)
    out[:, :, 0] = a5.transpose(0, 1, 3, 2)
    out[:, :, 1] = b5.transpose(0, 1, 3, 2)
    return out


def _make_runner():
    import jax
    from jax.sharding import Mesh, NamedSharding, PartitionSpec
    from jax.experimental.shard_map import shard_map
    from concourse.bass2jax import (
        _bass_exec_p, install_neuronx_cc_hook, partition_id_tensor,
    )

    nc = build_bass()
    install_neuronx_cc_hook()
    partition_name = nc.partition_id_tensor.name if nc.partition_id_tensor else None
    in_names, out_names, out_avals, zero_outs = [], [], [], []
    for alloc in nc.m.functions[0].allocations:
        if not isinstance(alloc, mybir.MemoryLocationSet):
            continue
        name = alloc.memorylocations[0].name
        if alloc.kind == "ExternalInput":
            if name != partition_name:
                in_names.append(name)
        elif alloc.kind == "ExternalOutput":
            shape = tuple(alloc.tensor_shape)
            dtype = mybir.dt.np(alloc.dtype)
            out_names.append(name)
            out_avals.append(jax.core.ShapedArray(shape, dtype))
            zero_outs.append(np.zeros(shape, dtype))
    n_params = len(in_names)
    all_in_names = in_names + out_names
    if partition_name is not None:
        all_in_names = all_in_names + [partition_name]

    def _body(*args):
        operands = list(args)
        if partition_name is not None:
            operands.append(partition_id_tensor())
        return tuple(_bass_exec_p.bind(
            *operands,
            out_avals=tuple(out_avals),
            in_names=tuple(all_in_names),
            out_names=tuple(out_names),
            lowering_input_output_aliases=(),
            sim_require_finite=True,
            sim_require_nnan=True,
            nc=nc,
        ))

    devices = jax.devices()[:NCORES]
    mesh = Mesh(np.asarray(devices), ("core",))
    in_specs = (PartitionSpec("core"),) * (n_params + len(out_names))
    out_specs = (PartitionSpec("core"),) * len(out_names)
    fn = jax.jit(
        shard_map(_body, mesh=mesh, in_specs=in_specs, out_specs=out_specs,
                  check_rep=False),
        keep_unused=True,
    )
    sharding = NamedSharding(mesh, PartitionSpec("core"))
    concat_zeros = [
        np.zeros((NCORES * z.shape[0], *z.shape[1:]), z.dtype) for z in zero_outs
    ]
    zeros_dev = [jax.device_put(z, sharding) for z in concat_zeros]

    def run(inputs_by_name):
        import jax as _jax
        args = [
            _jax.device_put(np.ascontiguousarray(inputs_by_name[nm]), sharding)
            for nm in in_names
        ]
        outs = fn(*args, *zeros_dev)
        return {nm: np.asarray(outs[i]) for i, nm in enumerate(out_names)}

    return run


def kernel(predictions_angle, targets_angle, predictions_distance, targets_distance):
    global _CACHED_RUNNER
    if _CACHED_RUNNER is None:
        _CACHED_RUNNER = _make_runner()
    out = _CACHED_RUNNER({
        "ang": _pack_pair(targets_angle, predictions_angle, NCH),
        "dst": _pack_pair(targets_distance, predictions_distance, NCH),
    })
    total = out["partials"].astype(np.float64).sum()
    return np.asarray(total / N / M, dtype=np.float32)
